# revision 1
# baseline (speedup 1.0000x reference)
"""BGK collision operator kernel for 8 Trainium2 NeuronCores.

omega[n,q] = (f_eq[n,q] - f[n,q]) / tau[n]

Key algebraic simplifications vs the reference:
  * The Newton solve has a closed form: the quadrature grid is uniform
    (xi_q ~= q*D), so the equilibrium weights are geometric with ratio
    r = v/(v+D), giving lam = ln(v/(v+D))/D and S0 = (v+D)/D.  The
    reference's 100 fixed Newton iterations converge to this same root
    (verified to ~3e-7 relative).
  * f_eq/tau is computed as a single exponential:
      f_eq[n,q]/tau[n] = exp(q*lam' + bias[n]),  lam' = ln(v)-ln(v+D)
      bias = ln(sum_q f) - ln(v+D) - (z+b4) + ln(D/64)
    where z+b4 is the MLP output (tau = exp(z+b4)).
  * r <= 1/(1+D) = 0.474, so exp terms for q >= 32 are < 5e-11 of scale
    and are exactly 0 at fp32 downstream: omega[:, 32:] = -f/tau.
"""

import numpy as np
from contextlib import ExitStack

import concourse.bass as bass
import concourse.tile as tile
from concourse import bacc, mybir
from concourse import bass_utils
from concourse.tile_rust import add_dep_helper

# ---------------------------------------------------------------- constants
N_FULL = 500000
Q = 64
QK = 32                  # quadrature points with non-negligible f_eq
NCORES = 8
TILE_ROWS = 4096         # rows per macro-tile (32 blocks of 128)
TILES_PER_CORE = 16
R_CORE = TILE_ROWS * TILES_PER_CORE          # 65536
N_PAD = R_CORE * NCORES                      # 524288

# exact fp32 bits of jnp.linspace(0, 70, 64) (differs from np.linspace in ulps)
XI = np.array([
    0x00000000, 0x3f8e38e4, 0x400e38e4, 0x40555556, 0x408e38e4, 0x40b1c71d,
    0x40d55556, 0x40f8e38f, 0x410e38e4, 0x41200000, 0x4131c71d, 0x41438e3a,
    0x41555556, 0x41671c72, 0x4178e38f, 0x41855556, 0x418e38e4, 0x41971c72,
    0x41a00000, 0x41a8e38f, 0x41b1c71d, 0x41baaaab, 0x41c38e3a, 0x41cc71c8,
    0x41d55556, 0x41de38e4, 0x41e71c72, 0x41f00001, 0x41f8e38f, 0x4200e38f,
    0x42055556, 0x4209c71d, 0x420e38e4, 0x4212aaab, 0x42171c72, 0x421b8e39,
    0x42200000, 0x422471c8, 0x4228e38f, 0x422d5556, 0x4231c71d, 0x423638e4,
    0x423aaaab, 0x423f1c72, 0x42438e3a, 0x42480001, 0x424c71c8, 0x4250e38f,
    0x42555556, 0x4259c71d, 0x425e38e4, 0x4262aaab, 0x42671c72, 0x426b8e3a,
    0x42700001, 0x427471c8, 0x4278e38f, 0x427d5556, 0x4280e38f, 0x42831c72,
    0x42855556, 0x42878e39, 0x4289c71d, 0x428c0000,
], dtype=np.uint32).view(np.float32)
DELTA = np.float64(70.0) / np.float64(63.0)
D32 = np.float32(DELTA)

F32 = mybir.dt.float32
AF = mybir.ActivationFunctionType
ALU = mybir.AluOpType


def _consts_array(Ws, bs):
    """Host-side consts tile [128, NC]: xiD, replicated biases, block-diag
    packed (pre-transposed) weights, and scalar bias columns."""
    W0, W1, W2, W3, W4 = Ws
    b0, b1, b2, b3, b4 = bs
    cols = {}
    c = np.zeros((128, 560), dtype=np.float32)
    # xiD[q] = xi_q / D for q < 32, replicated on all partitions
    xiD = (XI[:QK].astype(np.float64) / DELTA).astype(np.float32)
    c[:, 0:QK] = xiD[None, :]
    cols["xiD"] = (0, QK)
    # replicated hidden biases [b;b]
    for i, b in enumerate([b0, b1, b2, b3]):
        c[0:64, QK + i] = b
        c[64:128, QK + i] = b
        cols[f"b{i}r"] = (QK + i, QK + i + 1)
    o = QK + 4
    # lhsT0 [6, 128] block-diag of W0.T (W0 is [64, 3])
    c[0:3, o:o + 64] = W0.T
    c[3:6, o + 64:o + 128] = W0.T
    cols["lhsT0"] = (o, o + 128)
    o += 128
    for i, W in enumerate([W1, W2, W3]):
        c[0:64, o:o + 64] = W.T
        c[64:128, o + 64:o + 128] = W.T
        cols[f"lhsT{i + 1}"] = (o, o + 128)
        o += 128
    # lhsT4 [128, 2]
    c[0:64, o] = W4[0, :]
    c[64:128, o + 1] = W4[0, :]
    cols["lhsT4"] = (o, o + 2)
    o += 2
    # scalar columns (replicated across partitions)
    nb4 = np.float32(-float(b4[0]))
    CB = np.float32(np.log(DELTA / 64.0))
    c[:, o] = nb4
    cols["nb4"] = (o, o + 1)
    c[:, o + 1] = CB
    cols["CB"] = (o + 1, o + 2)
    o += 2
    return c[:, :o].copy(), cols


def build_nc(tiles_per_core=TILES_PER_CORE, nc_cols=None, repeat=1):
    rows = TILE_ROWS * tiles_per_core
    nc = bacc.Bacc("TRN2", target_bir_lowering=False, debug=False,
                   num_devices=NCORES)
    f_d = nc.dram_tensor("f", [rows, Q], F32, kind="ExternalInput").ap()
    xm_d = nc.dram_tensor("xm", [2, rows], F32, kind="ExternalInput").ap()
    xp_d = nc.dram_tensor("xp", [1, rows], F32, kind="ExternalInput").ap()
    cst_d = nc.dram_tensor("consts", [128, nc_cols], F32,
                           kind="ExternalInput").ap()
    out_d = nc.dram_tensor("out", [rows, Q], F32, kind="ExternalOutput").ap()

    with tile.TileContext(nc) as tc, ExitStack() as ctx:
        cpool = ctx.enter_context(tc.tile_pool(name="consts", bufs=1))
        main = ctx.enter_context(tc.tile_pool(name="main", bufs=3))
        hpool = ctx.enter_context(tc.tile_pool(name="h", bufs=6))
        zpool = ctx.enter_context(tc.tile_pool(name="zcols",
                                               bufs=min(tiles_per_core, 9)))
        opool = ctx.enter_context(tc.tile_pool(name="om", bufs=5))
        ppool = ctx.enter_context(tc.tile_pool(name="pk", bufs=5))
        # four independent 2-bank PSUM units; each MLP layer of a 2048-row
        # half-tile gets its own unit so consecutive half-tiles overlap on PE
        psu = [ctx.enter_context(tc.tile_pool(name=f"ps{u}", bufs=1,
                                              space="PSUM"))
               for u in range(4)]
        dpool = ctx.enter_context(tc.tile_pool(name="dram", bufs=2,
                                               space="DRAM"))

        cst = cpool.tile([128, nc_cols], F32)
        nc.sync.dma_start(cst[:], cst_d)

        def cc(name):
            a, b = build_nc.cols[name]
            return cst[:, a:b]

        last_act = [None]
        ACT_CHAIN = build_nc.act_chain

        def act_chain(inst):
            # serialize ACT ops in emission order so same-table-set
            # activations cluster (one table switch per cluster); other
            # engines stay unordered.
            if ACT_CHAIN and last_act[0] is not None:
                add_dep_helper(inst.ins, last_act[0].ins, False,
                               "act set clustering")
            last_act[0] = inst

        def mlp_body(i):
            # row mapping within a macro-tile: row = base + 32*p + j
            # (partition p holds 32 consecutive rows) -> every DMA below is
            # contiguous.
            base = i * TILE_ROWS
            # stacked x [6, 2048]: partitions 0-2 = (m0,m1,pos) of half A
            # rows (1024k..1024k+512), partitions 3-5 = half B rows; one
            # contiguous DMA per partition row.
            x_fm = main.tile([6, 2048], F32, tag="x_fm")
            xv = x_fm[:].rearrange("d (k c) -> d k c", k=4)
            for d, src_d, off in ((0, xm_d[0:1], 0), (1, xm_d[1:2], 0),
                                  (2, xp_d[0:1], 0), (3, xm_d[0:1], 512),
                                  (4, xm_d[1:2], 512), (5, xp_d[0:1], 512)):
                nc.sync.dma_start(
                    xv[d:d + 1],
                    src_d[:, base + off:base + off + 3584].rearrange(
                        "a (k c) -> a k c", c=512)[:, 0::2, :])
            # ---- MLP (feature-major, 2-subtile block-diag packing)
            # two 2048-row half-tiles emitted layer-interleaved: PE runs the
            # other half's matmuls during each tanh latency.  PSUM unit
            # u = 2*(li%2)+hf so exactly 8 banks stay live.
            hcur = [None, None]
            pscur = [None, None]
            for hf in range(2):
                co = 1024 * hf
                h_ps = psu[hf].tile([128, 1024], F32, tag=f"ps{hf}")
                for k in range(2):
                    nc.tensor.matmul(h_ps[:, 512 * k:512 * k + 512],
                                     cc("lhsT0")[0:6, :],
                                     x_fm[:, co + 512 * k:co + 512 * k + 512],
                                     start=True, stop=True)
                pscur[hf] = h_ps
            for li in (0, 1, 2, 3):
                for hf in range(2):
                    h = hpool.tile([128, 1024], F32, tag="h")
                    for k in range(2):
                        act_chain(nc.scalar.activation(
                            h[:, 512 * k:512 * k + 512],
                            pscur[hf][:, 512 * k:512 * k + 512], AF.Tanh,
                            bias=cc(f"b{li}r")))
                    hcur[hf] = h
                if li == 3:
                    break
                lw = cc(f"lhsT{li + 1}")
                for hf in range(2):
                    u = 2 * ((li + 1) % 2) + hf
                    h_ps = psu[u].tile([128, 1024], F32, tag=f"ps{u}")
                    for k in range(2):
                        nc.tensor.matmul(h_ps[:, 512 * k:512 * k + 512], lw,
                                         hcur[hf][:, 512 * k:512 * k + 512],
                                         start=True, stop=True)
                    pscur[hf] = h_ps
            z_halves = []
            for hf in range(2):
                z_ps = psu[hf].tile([128, 1024], F32, tag=f"ps{hf}")
                for k in range(2):
                    nc.tensor.matmul(z_ps[0:2, 512 * k:512 * k + 512],
                                     cc("lhsT4"),
                                     hcur[hf][:, 512 * k:512 * k + 512],
                                     start=True, stop=True)
                z_halves.append(z_ps)
            z_fm = main.tile([2, 2048], F32, tag="z_fm")
            for hf in range(2):
                nc.vector.tensor_copy(z_fm[:, 1024 * hf:1024 * hf + 1024],
                                      z_halves[hf][0:2, :])
            # bounce through DRAM to redistribute [2, 2048] -> [128, 32]:
            # store z into row order (row = 1024k + 512s + c), reload as
            # [128, 32] contiguous (row = 32p + j).
            zd = dpool.tile([1, TILE_ROWS], F32, tag="zd")
            nc.sync.dma_start(
                zd[:].rearrange("a (k s c) -> (a s) k c", k=4, s=2, c=512),
                z_fm[:].rearrange("s (k c) -> s k c", k=4))
            z_cols = zpool.tile([128, 32], F32, tag="z_cols")
            nc.sync.dma_start(
                z_cols[:], zd[:].rearrange("a (p j) -> (a p) j", p=128))
            return z_cols

        PHASE_FENCE = build_nc.phase_fence

        def eq_stage1(i, z_cols):
            base = i * TILE_ROWS
            f_t = main.tile([128, 32 * Q], F32, tag="f_t")
            nc.sync.dma_start(
                f_t[:],
                f_d[base:base + TILE_ROWS, :].rearrange(
                    "(p j) q -> p (j q)", p=128))
            pk = ppool.tile([128, 96], F32, tag="pk")
            nc.sync.dma_start(
                pk[:, 0:32],
                xm_d[0:1, base:base + TILE_ROWS].rearrange(
                    "d (p j) -> p (d j)", p=128))
            # negit = -exp(-(z + b4)) = -1/tau
            ng = main.tile([128, 32], F32, tag="ng")
            act_chain(nc.scalar.activation(ng[:], z_cols[:], AF.Exp,
                                           scale=-1.0, bias=cc("nb4")))
            nc.vector.tensor_scalar_mul(ng[:], ng[:], -1.0)
            # t = v + D
            nc.vector.tensor_scalar_add(pk[:, 32:64], pk[:, 0:32], float(D32))
            # omega[:, :] = f * negit ; acc_j = sum_q (f*negit)
            om = opool.tile([128, 32 * Q], F32, tag="om")
            acc = main.tile([128, 32], F32, tag="acc")
            for j in range(32):
                nc.vector.tensor_scalar(
                    om[:, Q * j:Q * j + Q], f_t[:, Q * j:Q * j + Q],
                    ng[:, j:j + 1], 0.0, ALU.mult, ALU.add,
                    accum_out=acc[:, j:j + 1])
            # pk[:,64:96] = -acc = (1/tau) * sum_q f
            nc.vector.tensor_scalar_mul(pk[:, 64:96], acc[:], -1.0)
            return pk, om

        def eq_stage2(pk):
            lnpk = ppool.tile([128, 96], F32, tag="lnpk")
            act_chain(nc.scalar.activation(lnpk[:], pk[:], AF.Ln))
            return lnpk

        def eq_stage3(i, lnpk, om):
            base = i * TILE_ROWS
            lamp = main.tile([128, 32], F32, tag="lamp")
            nc.vector.tensor_sub(lamp[:], lnpk[:, 0:32], lnpk[:, 32:64])
            bias0 = main.tile([128, 32], F32, tag="bias0")
            nc.vector.tensor_sub(bias0[:], lnpk[:, 64:96], lnpk[:, 32:64])
            # arg[p, j, q] = xiD[q] * lamp[p,j] + bias0[p,j]
            argt = main.tile([128, 32 * QK], F32, tag="argt")
            for j in range(32):
                nc.gpsimd.tensor_scalar(
                    argt[:, QK * j:QK * j + QK], cc("xiD"),
                    lamp[:, j:j + 1], bias0[:, j:j + 1], ALU.mult, ALU.add)
            wA = main.tile([128, 32 * QK], F32, tag="wA")
            act_chain(nc.scalar.activation(wA[:], argt[:], AF.Exp,
                                           bias=cc("CB")))
            # omega[:, :, :QK] += wA
            omv = om[:].rearrange("p (j q) -> p j q", j=32)
            wv = wA[:].rearrange("p (j q) -> p j q", j=32)
            nc.vector.tensor_add(omv[:, :, 0:QK], omv[:, :, 0:QK], wv)

            nc.sync.dma_start(
                out_d[base:base + TILE_ROWS, :].rearrange(
                    "(p j) q -> p (j q)", p=128),
                om[:])

        G = 4        # phase-B group size (ACT table-set clustering)
        BLK = 8      # phase interleave block (amortizes ACT table switches
                     # while overlapping one block's eq tail with the next
                     # block's MLP)

        def full_body():
            for b0 in range(0, tiles_per_core, BLK):
                bts = range(b0, min(b0 + BLK, tiles_per_core))
                zs = {i: mlp_body(i) for i in bts}
                if PHASE_FENCE:
                    tc.no_sync_barrier()
                for g0 in range(b0, b0 + len(bts), G):
                    gts = range(g0, min(g0 + G, b0 + len(bts)))
                    s1 = [eq_stage1(i, zs[i]) for i in gts]
                    s2 = [eq_stage2(pk) for pk, om in s1]
                    for i, lnpk, (pk, om) in zip(gts, s2, s1):
                        eq_stage3(i, lnpk, om)

        if repeat == 1:
            full_body()
        else:
            with tc.For_i(0, repeat, 1):
                full_body()

    nc.finalize()
    return nc


build_nc.cols = None
build_nc.act_chain = True
build_nc.phase_fence = False


def _prepare(f_distribution, macro_features, position_embedding, Ws, bs):
    consts, cols = _consts_array(Ws, bs)
    build_nc.cols = cols
    n = f_distribution.shape[0]
    f_pad = np.full((N_PAD, Q), 0.5, dtype=np.float32)
    f_pad[:n] = f_distribution
    xm_pad = np.full((2, N_PAD), 0.5, dtype=np.float32)
    xm_pad[:, :n] = macro_features.T
    xp_pad = np.zeros((1, N_PAD), dtype=np.float32)
    xp_pad[:, :n] = position_embedding.T
    in_maps = []
    for c in range(NCORES):
        sl = slice(c * R_CORE, (c + 1) * R_CORE)
        in_maps.append({
            "f": np.ascontiguousarray(f_pad[sl]),
            "xm": np.ascontiguousarray(xm_pad[:, sl]),
            "xp": np.ascontiguousarray(xp_pad[:, sl]),
            "consts": consts,
        })
    return in_maps, consts.shape[1]


def kernel(f_distribution, macro_features, position_embedding,
           W0, b0, W1, b1, W2, b2, W3, b3, W4, b4):
    f_distribution = np.ascontiguousarray(f_distribution, dtype=np.float32)
    macro_features = np.ascontiguousarray(macro_features, dtype=np.float32)
    position_embedding = np.ascontiguousarray(position_embedding,
                                              dtype=np.float32)
    Ws = [np.asarray(W, dtype=np.float32) for W in (W0, W1, W2, W3, W4)]
    bs = [np.asarray(b, dtype=np.float32) for b in (b0, b1, b2, b3, b4)]
    in_maps, ncols = _prepare(f_distribution, macro_features,
                              position_embedding, Ws, bs)
    nc = build_nc(TILES_PER_CORE, nc_cols=ncols)
    res = bass_utils.run_bass_kernel_spmd(nc, in_maps,
                                          core_ids=list(range(NCORES)))
    out = np.concatenate([res.results[c]["out"] for c in range(NCORES)],
                         axis=0)
    return out[:f_distribution.shape[0]]



# revision 10
# speedup vs baseline: 3.5431x; 3.5431x over previous
"""BGK collision operator kernel for 8 Trainium2 NeuronCores.

omega[n,q] = (f_eq[n,q] - f[n,q]) / tau[n]

Algebraic structure (vs the reference's Newton solve + exp/ln):
  * The quadrature grid is uniform (xi_q = q*D, D = 70/63), so the
    equilibrium weights are an exact geometric series:
      f_eq[q]/tau = c * r^q,  r = v/(v+D),
      c = rho * D/(v+D) / tau,  rho = mean_q f.
    The reference's Newton iteration converges to this same root.
  * r <= 1/(1+D) = 0.4737, so the q >= 16 tail is < 4e-6 of scale:
    omega[:, 16:] = -f/tau exactly (fp16 output resolution is coarser).
  * The geometric weights are built by a log-step multiply scan
    (w[2m:4m] = w[0:2m] * r^{2m}), so the only activation-table
    functions used are Tanh and Exp -- one table set, zero switches.

Layout: rows are tiled 4096 per macro-tile; within a tile row
r = 32*p + j (partition p holds 32 consecutive rows) so every f/omega
DMA moves 4KB contiguous per partition. The MLP packs two rows per
PE column (block-diagonal weights); fp16 operands run the PE at 4x
the fp32 rate. f and omega travel as fp16 (tolerance is 2e-2; fp16
element error is ~5e-4 of scale).
"""

import numpy as np
from contextlib import ExitStack

import concourse.bass as bass
import concourse.tile as tile
from concourse import bacc, mybir
from concourse import bass_utils

# ---------------------------------------------------------------- constants
N_FULL = 500000
Q = 64
QK = 16                  # quadrature points with non-negligible f_eq
NCORES = 8
TILE_ROWS = 4096
TILES_PER_CORE = 16
R_CORE = TILE_ROWS * TILES_PER_CORE          # 65536
N_PAD = R_CORE * NCORES                      # 524288

DELTA = np.float64(70.0) / np.float64(63.0)
D32 = np.float32(DELTA)
CD = float(np.float32(DELTA / 64.0))         # scale folded into w[...,0]

# exact fp32 bits of jnp.linspace(0, 70, 64) (kept for the test mirror)
XI = np.array([
    0x00000000, 0x3f8e38e4, 0x400e38e4, 0x40555556, 0x408e38e4, 0x40b1c71d,
    0x40d55556, 0x40f8e38f, 0x410e38e4, 0x41200000, 0x4131c71d, 0x41438e3a,
    0x41555556, 0x41671c72, 0x4178e38f, 0x41855556, 0x418e38e4, 0x41971c72,
    0x41a00000, 0x41a8e38f, 0x41b1c71d, 0x41baaaab, 0x41c38e3a, 0x41cc71c8,
    0x41d55556, 0x41de38e4, 0x41e71c72, 0x41f00001, 0x41f8e38f, 0x4200e38f,
    0x42055556, 0x4209c71d, 0x420e38e4, 0x4212aaab, 0x42171c72, 0x421b8e39,
    0x42200000, 0x422471c8, 0x4228e38f, 0x422d5556, 0x4231c71d, 0x423638e4,
    0x423aaaab, 0x423f1c72, 0x42438e3a, 0x42480001, 0x424c71c8, 0x4250e38f,
    0x42555556, 0x4259c71d, 0x425e38e4, 0x4262aaab, 0x42671c72, 0x426b8e3a,
    0x42700001, 0x427471c8, 0x4278e38f, 0x427d5556, 0x4280e38f, 0x42831c72,
    0x42855556, 0x42878e39, 0x4289c71d, 0x428c0000,
], dtype=np.uint32).view(np.float32)

F32 = mybir.dt.float32
F16 = mybir.dt.float16
AF = mybir.ActivationFunctionType
ALU = mybir.AluOpType
AXL = mybir.AxisListType

# consts column layout
C32_COLS = {"b0r": 0, "b1r": 1, "b2r": 2, "b3r": 3, "nb4": 4}
NC32 = 5
C16_COLS = {"lhsT0": (0, 128), "lhsT1": (128, 256), "lhsT2": (256, 384),
            "lhsT3": (384, 512),
            "lhsT4_0": (512, 520), "lhsT4_1": (520, 528),
            "lhsT4_2": (528, 536), "lhsT4_3": (536, 544)}
NC16 = 544


def _consts_arrays(Ws, bs):
    """Host-side consts: c32 [128,5] biases, c16 [128,514] packed weights."""
    W0, W1, W2, W3, W4 = Ws
    b0, b1, b2, b3, b4 = bs
    c32 = np.zeros((128, NC32), dtype=np.float32)
    for i, b in enumerate([b0, b1, b2, b3]):
        c32[0:64, i] = b
        c32[64:128, i] = b
    c32[:, 4] = np.float32(-float(b4[0]))
    c16 = np.zeros((128, NC16), dtype=np.float16)
    # lhsT0 [6, 128] block-diag of W0.T (W0 is [64, 3])
    c16[0:3, 0:64] = W0.T
    c16[3:6, 64:128] = W0.T
    for i, W in enumerate([W1, W2, W3]):
        a = 128 * (i + 1)
        c16[0:64, a:a + 64] = W.T
        c16[64:128, a + 64:a + 128] = W.T
    # lhsT4 variant v = 2*hf + kk: [128, 8] with W4 only in columns
    # 2v+b (slot b at partitions 64b..64b+63); the four z matmuls
    # accumulate into one [8, 512] PSUM tile.
    for v in range(4):
        a = 512 + 8 * v
        c16[0:64, a + 2 * v] = W4[0, :]
        c16[64:128, a + 2 * v + 1] = W4[0, :]
    return c32, c16


def build_nc(tiles_per_core=TILES_PER_CORE, repeat=1):
    rows = TILE_ROWS * tiles_per_core
    nc = bacc.Bacc("TRN2", target_bir_lowering=False, debug=False,
                   num_devices=NCORES)
    f_d = nc.dram_tensor("f", [rows, Q], F16, kind="ExternalInput").ap()
    x6_d = nc.dram_tensor("x6", [6, rows // 2], F16, kind="ExternalInput").ap()
    v_d = nc.dram_tensor("v", [128, 32 * tiles_per_core], F32,
                         kind="ExternalInput").ap()
    c32_d = nc.dram_tensor("c32", [128, NC32], F32, kind="ExternalInput").ap()
    c16_d = nc.dram_tensor("c16", [128, NC16], F16, kind="ExternalInput").ap()
    out_d = nc.dram_tensor("out", [rows, Q], F16, kind="ExternalOutput").ap()

    with tile.TileContext(nc) as tc, ExitStack() as ctx:
        cp32 = ctx.enter_context(tc.tile_pool(name="c32", bufs=1))
        cp16 = ctx.enter_context(tc.tile_pool(name="c16", bufs=1))
        vpool = ctx.enter_context(tc.tile_pool(name="v", bufs=1))
        xpool = ctx.enter_context(tc.tile_pool(name="x", bufs=3))
        fpool = ctx.enter_context(tc.tile_pool(name="f", bufs=6))
        hpool = ctx.enter_context(tc.tile_pool(name="h", bufs=6))
        zpool = ctx.enter_context(tc.tile_pool(name="z", bufs=6))
        npool = ctx.enter_context(tc.tile_pool(name="ng", bufs=6))
        spool = ctx.enter_context(tc.tile_pool(name="sm", bufs=14))
        gpool = ctx.enter_context(tc.tile_pool(name="gr", bufs=10))
        wpool = ctx.enter_context(tc.tile_pool(name="w", bufs=4))
        opool = ctx.enter_context(tc.tile_pool(name="om", bufs=6))
        psu = [ctx.enter_context(tc.tile_pool(name=f"ps{u}", bufs=1,
                                              space="PSUM"))
               for u in range(4)]
        dpool = ctx.enter_context(tc.tile_pool(name="dram", bufs=2,
                                               space="DRAM"))

        cst32 = cp32.tile([128, NC32], F32)
        nc.sync.dma_start(cst32[:], c32_d)
        cst16 = cp16.tile([128, NC16], F16)
        nc.sync.dma_start(cst16[:], c16_d)

        def cc32(name):
            a = C32_COLS[name]
            return cst32[:, a:a + 1]

        def cc16(name):
            a, b = C16_COLS[name]
            return cst16[:, a:b]

        def mlp_body(i):
            # x_fm [6, 2048]: partition 3b+d = feature d of block-slot b;
            # column 512*K + c = row 1024*K + 512*b + c of the tile
            # (host pre-arranged x6 so this is one contiguous DMA).
            x_fm = xpool.tile([6, 2048], F16, tag="x")
            nc.sync.dma_start(x_fm[:], x6_d[:, 2048 * i:2048 * i + 2048])
            f_t = fpool.tile([128, 2048], F16, tag="f")
            nc.sync.dma_start(
                f_t[:],
                f_d[4096 * i:4096 * i + 4096, :].rearrange(
                    "(p j) q -> p (j q)", p=128))
            acc = spool.tile([128, 32], F32, tag="acc")
            nc.vector.tensor_reduce(
                acc[:], f_t[:].rearrange("p (j q) -> p j q", j=32),
                axis=AXL.X, op=ALU.add)
            # ---- MLP: two 1024-col halves, layer-interleaved so PE covers
            # each tanh latency with the other half's matmuls.
            hcur = [None, None]
            pscur = [None, None]
            for hf in range(2):
                ps = psu[hf].tile([128, 1024], F32, tag=f"ps{hf}")
                for k in range(2):
                    nc.tensor.matmul(
                        ps[:, 512 * k:512 * k + 512], cc16("lhsT0")[0:6, :],
                        x_fm[:, 1024 * hf + 512 * k:1024 * hf + 512 * k + 512],
                        start=True, stop=True)
                pscur[hf] = ps
            for li in range(4):
                for hf in range(2):
                    h = hpool.tile([128, 1024], F16, tag="h")
                    nc.scalar.activation(h[:], pscur[hf][:], AF.Tanh,
                                         bias=cc32(f"b{li}r"))
                    hcur[hf] = h
                if li == 3:
                    break
                lw = cc16(f"lhsT{li + 1}")
                for hf in range(2):
                    u = 2 * ((li + 1) % 2) + hf
                    ps = psu[u].tile([128, 1024], F32, tag=f"ps{u}")
                    for k in range(2):
                        nc.tensor.matmul(ps[:, 512 * k:512 * k + 512], lw,
                                         hcur[hf][:, 512 * k:512 * k + 512],
                                         start=True, stop=True)
                    pscur[hf] = ps
            # layer 4 -> z packed [8, 512] in one PSUM bank (partition
            # offset 4*hf+2*kk selects the slot), one cheap 512-col copy to
            # SBUF, then one DMA shuffles feature-major z into row-major
            # z_cols [128, 32].
            # z accumulates into bank 1 of half-0's (already-consumed) l3
            # psum tile -- no extra PSUM needed.
            z_cols = zpool.tile([128, 32], F32, tag="z")
            zps = pscur[0][0:8, 512:1024]
            for v, (hf, k) in enumerate([(0, 0), (0, 1), (1, 0), (1, 1)]):
                nc.tensor.matmul(zps, cc16(f"lhsT4_{v}"),
                                 hcur[hf][:, 512 * k:512 * k + 512],
                                 start=(v == 0), stop=(v == 3))
            z_fm = zpool.tile([8, 512], F32, tag="zfm")
            nc.vector.tensor_copy(z_fm[:], zps)
            # partition P = 4hf+2kk+b, so P*512 + 32t + j IS the row index:
            # the DRAM bounce lands in row order and reloads row-major.
            zd = dpool.tile([1, TILE_ROWS], F32, tag="zd")
            nc.gpsimd.dma_start(
                zd[0:1, :].rearrange("a (P c) -> (a P) c", P=8), z_fm[:])
            nc.gpsimd.dma_start(
                z_cols[:], zd[0:1, :].rearrange("a (p j) -> (a p) j", p=128))
            return f_t, acc, z_cols

        def group_prep(v_t, g0, ngt):
            # per-group scalars from v: u = 1/(v+D), r = v*u, r^2, r^4, r^8
            W = 32 * ngt
            sl = v_t[:, 32 * g0:32 * g0 + W]
            tD = gpool.tile([128, W], F32, tag="tD")
            nc.vector.tensor_scalar_add(tD[:], sl, float(D32))
            u = gpool.tile([128, W], F32, tag="u")
            nc.vector.reciprocal(u[:], tD[:])
            r = gpool.tile([128, W], F32, tag="r")
            nc.vector.tensor_mul(r[:], sl, u[:])
            rp2 = gpool.tile([128, W], F32, tag="rp2")
            nc.vector.tensor_mul(rp2[:], r[:], r[:])
            rp4 = gpool.tile([128, W], F32, tag="rp4")
            nc.vector.tensor_mul(rp4[:], rp2[:], rp2[:])
            rp8 = gpool.tile([128, W], F32, tag="rp8")
            nc.vector.tensor_mul(rp8[:], rp4[:], rp4[:])
            return u, r, rp2, rp4, rp8

        def eq_tile(i, ti, f_t, acc, z_cols, grp):
            u, r, rp2, rp4, rp8 = grp
            ng = npool.tile([128, 32], F32, tag="ng")
            nc.scalar.activation(ng[:], z_cols[:], AF.Exp,
                                 scale=-1.0, bias=cc32("nb4"))
            # omega = -f * (1/tau) everywhere (the q < QK equilibrium part
            # is added below)
            om = opool.tile([128, 2048], F16, tag="om")
            fv = f_t[:].rearrange("p (j q) -> p j q", j=32)
            omv = om[:].rearrange("p (j q) -> p j q", j=32)
            ngb = ng[:].unsqueeze(2).broadcast_to([128, 32, 64])
            nc.gpsimd.scalar_tensor_tensor(omv, fv, -1.0, ngb,
                                           op0=ALU.mult, op1=ALU.mult)
            # c = rho * 64 * ng * u * D/64 folded: c2 = acc*ng*u, then *D/64
            c1 = spool.tile([128, 32], F32, tag="c1")
            nc.vector.tensor_mul(c1[:], acc[:], ng[:])
            c2 = spool.tile([128, 32], F32, tag="c2")
            nc.vector.tensor_mul(c2[:], c1[:], u[:, 32 * ti:32 * ti + 32])
            # geometric scan: w[j, q] = c * r^q for q < QK
            w = wpool.tile([128, 32 * QK], F32, tag="w")
            wv = w[:].rearrange("p (j q) -> p j q", j=32)
            nc.vector.tensor_scalar_mul(wv[:, :, 0:1], c2[:].unsqueeze(2), CD)

            def rs(t, m):
                return t[:, 32 * ti:32 * ti + 32].unsqueeze(2).broadcast_to(
                    [128, 32, m])

            nc.vector.tensor_mul(wv[:, :, 1:2], wv[:, :, 0:1], rs(r, 1))
            nc.vector.tensor_mul(wv[:, :, 2:4], wv[:, :, 0:2], rs(rp2, 2))
            nc.vector.tensor_mul(wv[:, :, 4:8], wv[:, :, 0:4], rs(rp4, 4))
            nc.vector.tensor_mul(wv[:, :, 8:16], wv[:, :, 0:8], rs(rp8, 8))
            nc.vector.tensor_add(omv[:, :, 0:QK], omv[:, :, 0:QK], wv[:])
            nc.gpsimd.dma_start(
                out_d[4096 * i:4096 * i + 4096, :].rearrange(
                    "(p j) q -> p (j q)", p=128),
                om[:])

        def full_body():
            v_t = vpool.tile([128, 32 * tiles_per_core], F32, tag="v")
            nc.sync.dma_start(v_t[:], v_d)
            for g0 in range(0, tiles_per_core, 4):
                gts = list(range(g0, min(g0 + 4, tiles_per_core)))
                st = {i: mlp_body(i) for i in gts}
                grp = group_prep(v_t, g0, len(gts))
                for ti, i in enumerate(gts):
                    f_t, acc, z_cols = st[i]
                    eq_tile(i, ti, f_t, acc, z_cols, grp)

        if repeat == 1:
            full_body()
        else:
            with tc.For_i(0, repeat, 1):
                full_body()

    nc.finalize()
    return nc


def _prepare(f_distribution, macro_features, position_embedding, Ws, bs):
    c32, c16 = _consts_arrays(Ws, bs)
    n = f_distribution.shape[0]
    f16 = np.full((N_PAD, Q), 0.5, dtype=np.float16)
    f16[:n] = f_distribution
    x3 = np.full((3, N_PAD), 0.5, dtype=np.float16)
    x3[0, :n] = macro_features[:, 0]
    x3[1, :n] = macro_features[:, 1]
    x3[2, :n] = position_embedding[:, 0]
    x3[2, n:] = 0.0
    v = np.full((N_PAD,), 0.5, dtype=np.float32)
    v[:n] = macro_features[:, 0]
    T = TILES_PER_CORE
    in_maps = []
    for c in range(NCORES):
        sl = slice(c * R_CORE, (c + 1) * R_CORE)
        x6 = np.ascontiguousarray(
            x3[:, sl].reshape(3, T, 4, 2, 512).transpose(3, 0, 1, 2, 4)
            .reshape(6, R_CORE // 2))
        vc = np.ascontiguousarray(
            v[sl].reshape(T, 128, 32).transpose(1, 0, 2).reshape(128, 32 * T))
        in_maps.append({
            "f": np.ascontiguousarray(f16[sl]),
            "x6": x6,
            "v": vc,
            "c32": c32,
            "c16": c16,
        })
    return in_maps


def kernel(f_distribution, macro_features, position_embedding,
           W0, b0, W1, b1, W2, b2, W3, b3, W4, b4):
    f_distribution = np.ascontiguousarray(f_distribution, dtype=np.float32)
    macro_features = np.ascontiguousarray(macro_features, dtype=np.float32)
    position_embedding = np.ascontiguousarray(position_embedding,
                                              dtype=np.float32)
    Ws = [np.asarray(W, dtype=np.float32) for W in (W0, W1, W2, W3, W4)]
    bs = [np.asarray(b, dtype=np.float32) for b in (b0, b1, b2, b3, b4)]
    in_maps = _prepare(f_distribution, macro_features,
                       position_embedding, Ws, bs)
    nc = build_nc(TILES_PER_CORE)
    res = bass_utils.run_bass_kernel_spmd(nc, in_maps,
                                          core_ids=list(range(NCORES)))
    out = np.concatenate([res.results[c]["out"] for c in range(NCORES)],
                         axis=0)
    return out[:f_distribution.shape[0]].astype(np.float32)
